# revision 15
# baseline (speedup 1.0000x reference)
"""GCN 2-layer encoder on 8 Trainium2 NeuronCores (Bass/Tile).

Math (PyG GCNConv x2, self-loops, symmetric norm):
    A' = A + I, deg = indegree(A'), dinv = deg^-1/2
    h1 = relu(dinv * (A' (dinv * (x W1))) + b1)
    out = dinv * (A' (dinv * (h1 W2))) + b2

Sharding: dst nodes split contiguously across 8 cores (12500 each). Each
core projects its own rows (x W) in bf16; an AllGather builds the full
projected table (bf16) in DRAM; each core aggregates its dst rows by
gathering per-edge source rows with the DMAGatherAnt extended instruction
(thousands of int16 indices per instruction) and reducing per group with
strided DVE tensor_reduce.

dma_gather takes int16 indices with a 256-byte row pitch, so table rows
(64B bf16) are addressed as v = 4q + s: four "residue classes" s with
shifted base views (offset s*32 elements, row stride 128 elements = 256B)
and q = v//4 <= 25087 fits int16. Slots are laid out per (group, class)
with width = max per-partition count (padding points at an all-zero row).

Host prep: nodes per core are degree-sorted into groups of 128; residues
within each sorted 4-tuple are rebalanced (ICM sweeps) to even class
counts per destination list, shrinking padded widths. All widths are
maxed across cores so all 8 cores run one identical SPMD program.
"""

import numpy as np

N = 100000
IN_C, HID, OUT_C = 256, 32, 16
NCORES = 8
P = 128
NPC = N // NCORES            # nodes per core: 12500
NGROUP = (NPC + P - 1) // P  # 98 groups
NPAD = NGROUP * P            # 12544 rows per core slice (incl. dummies)
VTOT = NCORES * NPAD         # table rows: 100352
NCLS = 4
NQ = VTOT // NCLS            # q-space size: 25088
GCH = 6                      # groups per gather chunk
TCH = 4                      # groups per transpose/proj batch


def _rebalance_residues(perms, src, dst, deg, iters=10):
    """Damped ICM sweeps: permute nodes within each degree-sorted 4-tuple so
    that every destination list has near-balanced residue-class counts.
    Only a random subset of tuples updates per sweep (herding otherwise)."""
    prng = np.random.default_rng(7)
    # position-in-core (row) for each node; tuples are rows 4t..4t+3
    pos = np.empty(N, dtype=np.int64)
    for c in range(NCORES):
        pos[perms[c]] = np.arange(NPC)
    node_core = (np.arange(N) // NPC).astype(np.int64)
    s_of = (pos % 4).astype(np.int64)        # current class per node
    tup = pos // 4                            # tuple id within core
    tup_g = node_core * (NPC // 4) + tup      # global tuple id
    ntup = NCORES * (NPC // 4)

    # tuple member table [ntup, 4]: node with s_of == k
    members = np.empty((ntup, 4), dtype=np.int64)
    members[tup_g, s_of] = np.arange(N)

    from itertools import permutations
    PERMS = np.array(list(permutations(range(4))), dtype=np.int64)  # [24, 4]

    outdeg = np.bincount(src, minlength=N).astype(np.int64)
    for _ in range(iters):
        cnt = np.zeros((N, NCLS), dtype=np.int32)
        np.add.at(cnt, (dst, s_of[src]), 1)
        # score[v, s]: load v adds to class s over its out-neighbors
        score = np.zeros((N, NCLS), dtype=np.int64)
        np.add.at(score, src, cnt[dst].astype(np.int64))
        score[np.arange(N), s_of] -= outdeg
        # member scores [ntup, 4 slots, 4 classes]
        ms = score[members]                   # [ntup, 4, 4]
        # cost of each of 24 assignments: slot i -> class PERMS[p, i]
        cost = ms[:, np.arange(4)[None, :], PERMS].sum(axis=2)  # [ntup, 24]
        best = PERMS[np.argmin(cost, axis=1)]  # [ntup, 4]
        upd = prng.random(ntup) < 0.3
        sel = members[upd]
        s_of[sel.ravel()] = best[upd].ravel()

    # rebuild perms: node at row 4t + s_of within its core
    newpos = tup * 4 + s_of
    newperms = []
    for c in range(NCORES):
        nodes = np.arange(c * NPC, (c + 1) * NPC)
        pr = np.empty(NPC, dtype=np.int64)
        pr[newpos[nodes]] = nodes
        newperms.append(pr)
    return newperms


def _host_prep(x, edge_index, W1, b1, W2, b2):
    import ml_dtypes
    bf16 = ml_dtypes.bfloat16

    x = np.asarray(x, dtype=np.float32)
    ei = np.asarray(edge_index)
    W1 = np.asarray(W1, dtype=np.float32)
    b1 = np.asarray(b1, dtype=np.float32)
    W2 = np.asarray(W2, dtype=np.float32)
    b2 = np.asarray(b2, dtype=np.float32)

    loops = np.arange(N, dtype=np.int64)
    src = np.concatenate([ei[0], loops]).astype(np.int64)
    dst = np.concatenate([ei[1], loops]).astype(np.int64)

    deg = np.bincount(dst, minlength=N).astype(np.int64)
    dinv = (1.0 / np.sqrt(np.maximum(deg, 1))).astype(np.float32)

    perms = []
    for c in range(NCORES):
        nodes = np.arange(c * NPC, (c + 1) * NPC)
        perms.append(nodes[np.argsort(deg[nodes], kind="stable")])
    perms = _rebalance_residues(perms, src, dst, deg)

    core_of = (np.arange(N) // NPC).astype(np.int64)
    pos_in_core = np.empty(N, dtype=np.int64)
    for c in range(NCORES):
        pos_in_core[perms[c]] = np.arange(NPC)
    rowid = core_of * NPAD + pos_in_core  # global table row of each node

    s_row = (rowid % NCLS).astype(np.int64)
    q_row = (rowid // NCLS).astype(np.int64)

    # zero rows per class (core-0 dummy rows NPC..NPC+3 are all-zero)
    qzero = np.empty(NCLS, dtype=np.int64)
    for r in range(NPC, NPC + NCLS):
        qzero[r % NCLS] = r // NCLS

    # per-core per-(group,row,class) counts -> widths maxed across cores
    cnt_all = []
    Wgs = np.zeros((NGROUP, NCLS), dtype=np.int64)
    for c in range(NCORES):
        m = (rowid[dst] // NPAD) == c
        drow = rowid[dst[m]] % NPAD
        g = drow // P
        p = drow % P
        se = s_row[src[m]]
        cnt = np.zeros((NGROUP, P, NCLS), dtype=np.int32)
        np.add.at(cnt, (g, p, se), 1)
        cnt_all.append((m, cnt))
        Wgs = np.maximum(Wgs, cnt.max(axis=1))
    Wgs = np.maximum(Wgs, 0)

    # gather chunks: greedy group runs, capped by per-class instruction size
    # (Q7 scratch caps num_idxs ~16k -> class cols <= 120) and SBUF width
    chunks = []
    a = 0
    while a < NGROUP:
        b = a + 1
        while b < NGROUP and b - a < 8:
            cc = [int(Wgs[a:b + 1, s].sum()) for s in range(NCLS)]
            if max(cc) > 120 or sum(cc) > 520:
                break
            b += 1
        chunks.append((a, b))
        a = b
    # per chunk: class col counts and offsets
    layout = []
    idx_cols_total = 0
    for a, b in chunks:
        ccols = [int(Wgs[a:b, s].sum()) for s in range(NCLS)]
        total = sum(ccols)
        # col offset of (g, s) segment within the chunk tile:
        offs = {}
        acc = 0
        for s in range(NCLS):
            for g in range(a, b):
                offs[(g, s)] = acc
                acc += int(Wgs[g, s])
        # idx block col offset (in int16 words per partition) per class
        m_per_cls = [128 * cc // 16 for cc in ccols]
        layout.append(dict(a=a, b=b, ccols=ccols, total=total, offs=offs,
                           m=m_per_cls, mstart=idx_cols_total))
        idx_cols_total += sum(m_per_cls)

    # build per-core idx arrays [128, idx_cols_total] int16
    idx_list = []
    for c in range(NCORES):
        m, cnt = cnt_all[c]
        drow = rowid[dst[m]] % NPAD
        g = drow // P
        p = drow % P
        se = s_row[src[m]]
        qe = q_row[src[m]]
        # rank within (g, p, s)
        key = (g * P + p) * NCLS + se
        order = np.argsort(key, kind="stable")
        ks = key[order]
        qs = qe[order]
        cum = np.concatenate(
            [[0], np.cumsum(np.bincount(ks, minlength=NGROUP * P * NCLS))])
        j = np.arange(ks.size) - cum[ks]
        gg = ks // (P * NCLS)
        pp = (ks // NCLS) % P
        ss = ks % NCLS

        idx_arr = np.empty((P, idx_cols_total), dtype=np.int16)
        # fill with per-class zero-row q, then scatter real q values
        ch_of_g = np.empty(NGROUP, dtype=np.int64)
        for li, L in enumerate(layout):
            ch_of_g[L["a"]:L["b"]] = li
        # per chunk fill pads
        for li, L in enumerate(layout):
            base = L["mstart"]
            for s in range(NCLS):
                Mw = L["m"][s]
                idx_arr[:, base:base + Mw] = np.int16(qzero[s])
                base += Mw
        # compute list position for each edge
        li_arr = ch_of_g[gg]
        Ls = [layout[int(x)] for x in range(len(layout))]
        mstart = np.array([L["mstart"] for L in Ls], dtype=np.int64)
        # class block start within chunk (in idx cols)
        clsoff = np.zeros((len(layout), NCLS), dtype=np.int64)
        for li, L in enumerate(Ls):
            acc = 0
            for s in range(NCLS):
                clsoff[li, s] = acc
                acc += L["m"][s]
        # column within class block
        segoff = np.zeros((NGROUP, NCLS), dtype=np.int64)
        for li, L in enumerate(Ls):
            for s in range(NCLS):
                acc = 0
                for g in range(L["a"], L["b"]):
                    segoff[g, s] = acc
                    acc += int(Wgs[g, s])
        col = segoff[gg, ss] + j          # column within class instruction
        k = col * P + pp                  # list position
        wcol = mstart[li_arr] + clsoff[li_arr, ss] + (k // 16)
        wrow = k % 16
        idx_arr[wrow, wcol] = qs.astype(np.int16)
        # replicate the 16-row wrap to all 128 partitions
        for r in range(1, 8):
            idx_arr[16 * r:16 * (r + 1), :] = idx_arr[0:16, :]
        idx_list.append(idx_arr)

    # per-core dense inputs
    xT_list, dinv_list = [], []
    for c in range(NCORES):
        xp = np.zeros((NPAD, IN_C), dtype=np.float32)
        xp[:NPC] = x[perms[c]]
        xT_list.append(np.ascontiguousarray(xp.T).astype(bf16))
        dv = np.zeros(NPAD, dtype=np.float32)
        dv[:NPC] = dinv[perms[c]]
        dinv_list.append(np.ascontiguousarray(dv.reshape(NGROUP, P).T))

    b1b = np.tile(b1[None, :], (P, 1)).astype(np.float32)
    b2b = np.tile(b2[None, :], (P, 1)).astype(np.float32)

    lay_key = (tuple(int(w) for w in Wgs.ravel()), idx_cols_total)
    build = dict(Wgs=Wgs, layout=layout, idx_cols_total=idx_cols_total,
                 key=lay_key)
    return dict(build=build, idx=idx_list, xT=xT_list, dinv=dinv_list,
                W1=W1.astype(bf16), W2=W2.astype(bf16), b1b=b1b, b2b=b2b,
                perms=perms)


_NC_CACHE = {}


def _build_bass(build, repeat=1):
    key = (build["key"], repeat)
    if key in _NC_CACHE:
        return _NC_CACHE[key]

    import concourse.bacc as bacc
    import concourse.bass as bass
    import concourse.tile as tile
    import concourse.mybir as mybir
    from concourse.masks import make_identity

    f32 = mybir.dt.float32
    bf16 = mybir.dt.bfloat16
    i16 = mybir.dt.int16

    Wgs = build["Wgs"]
    layout = build["layout"]
    idx_cols_total = build["idx_cols_total"]

    nc = bacc.Bacc("TRN2", target_bir_lowering=False, debug=False,
                   num_devices=NCORES)

    xT_t = nc.dram_tensor("xT", [IN_C, NPAD], bf16, kind="ExternalInput")
    idx_t = nc.dram_tensor("idx", [P, idx_cols_total], i16,
                           kind="ExternalInput")
    dinv_t = nc.dram_tensor("dinv", [P, NGROUP], f32, kind="ExternalInput")
    W1_t = nc.dram_tensor("W1", [IN_C, HID], bf16, kind="ExternalInput")
    W2_t = nc.dram_tensor("W2", [HID, OUT_C], bf16, kind="ExternalInput")
    b1b_t = nc.dram_tensor("b1b", [P, HID], f32, kind="ExternalInput")
    b2b_t = nc.dram_tensor("b2b", [P, OUT_C], f32, kind="ExternalInput")
    out_t = nc.dram_tensor("out", [NPAD, OUT_C], f32, kind="ExternalOutput")

    hs1_own = nc.dram_tensor("hs1_own", [NPAD, HID], bf16)
    hs2_own = nc.dram_tensor("hs2_own", [NPAD, HID], bf16)  # 16 real cols
    table1 = nc.dram_tensor("table1", [VTOT, HID], bf16, addr_space="Shared")
    table2 = nc.dram_tensor("table2", [VTOT, HID], bf16, addr_space="Shared")

    rg = [list(range(NCORES))]
    PCH = 8
    pchunks = [(a, min(a + PCH, NGROUP)) for a in range(0, NGROUP, PCH)]

    relu = mybir.ActivationFunctionType.Relu
    add = mybir.AluOpType.add
    AX = mybir.AxisListType.X

    WMAXCH = max(L["total"] for L in layout)
    MMAXCH = max(sum(L["m"]) for L in layout)

    def emit_gather(out_ap, in_ap, idxs_ap, num_idxs, elem_size):
        gp = nc.gpsimd
        elem_step = in_ap.ap[0][0]
        stride_bytes = elem_step * mybir.dt.size(in_ap.dtype)
        assert stride_bytes % 256 == 0 and stride_bytes // 256 < 256
        _in = gp.lower_ap_dma(in_ap, for_custom_bir_dma=True)
        _idx = gp.lower_ap(idxs_ap)
        _out = gp.lower_ap(out_ap)
        return gp.add_instruction(mybir.InstDMAGatherAnt(
            name=gp.bass.get_next_instruction_name(),
            ins=[*_in, _idx, gp.lower_val_access(gp.to_reg(num_idxs))],
            outs=[_out], transpose=False, num_idxs=num_idxs,
            elem_size=elem_size, stride_bytes_256=stride_bytes // 256,
            gen_mode=0, single_packet=False, queue_num=0,
            sbuf_tokens_per_rank=0, sbuf_free_dim_per_rank=0,
            sbuf_free_dim_pad_per_rank=0, sbuf_byte_offset=0))

    with tile.TileContext(nc) as tc:
        with tc.tile_pool(name="const", bufs=1) as cp, \
             tc.tile_pool(name="xt", bufs=3) as xp, \
             tc.tile_pool(name="gat", bufs=2) as gp_pool, \
             tc.tile_pool(name="idxp", bufs=2) as ixp, \
             tc.tile_pool(name="work", bufs=4) as wp, \
             tc.tile_pool(name="h1p", bufs=2) as h1p, \
             tc.tile_pool(name="ps1", bufs=2, space="PSUM") as ps1, \
             tc.tile_pool(name="psT", bufs=2, space="PSUM") as psT, \
             tc.tile_pool(name="ps2", bufs=2, space="PSUM") as ps2:

            ident = cp.tile([P, P], bf16)
            make_identity(nc, ident[:])
            w1a = cp.tile([P, HID], bf16)
            w1b = cp.tile([P, HID], bf16)
            nc.sync.dma_start(out=w1a[:], in_=W1_t[0:P, :])
            nc.sync.dma_start(out=w1b[:], in_=W1_t[P:IN_C, :])
            w2blk = cp.tile([P, TCH * OUT_C], bf16)
            nc.gpsimd.memset(w2blk[:], 0.0)
            for _k in range(TCH):
                nc.sync.dma_start(
                    out=w2blk[_k * HID:(_k + 1) * HID,
                              _k * OUT_C:(_k + 1) * OUT_C],
                    in_=W2_t[:, :])
            b1s = cp.tile([P, HID], f32)
            nc.sync.dma_start(out=b1s[:], in_=b1b_t[:, :])
            b2s = cp.tile([P, OUT_C], f32)
            nc.sync.dma_start(out=b2s[:], in_=b2b_t[:, :])
            dvs = cp.tile([P, NGROUP], f32)
            nc.sync.dma_start(out=dvs[:], in_=dinv_t[:, :])

            tab1v = table1[:, :].rearrange("(q s) h -> q s h", s=NCLS)
            tab2v = table2[:, :].rearrange("(q s) h -> q s h", s=NCLS)

            for _rep in range(repeat):
              # ---- P1: project own rows (bf16) into hs1_own ----
              for a, b in pchunks:
                  nb = b - a
                  xt0 = xp.tile([P, PCH * P], bf16, tag="xt0")
                  xt1 = xp.tile([P, PCH * P], bf16, tag="xt1")
                  nc.sync.dma_start(out=xt0[:, 0:nb * P],
                                    in_=xT_t[0:P, a * P:b * P])
                  nc.sync.dma_start(out=xt1[:, 0:nb * P],
                                    in_=xT_t[P:IN_C, a * P:b * P])
                  hs1c = wp.tile([P, PCH * HID], bf16, tag="hs1c")
                  for gi in range(nb):
                      g = a + gi
                      pm = ps1.tile([P, HID], f32, tag="pm1")
                      nc.tensor.matmul(out=pm[:],
                                       lhsT=xt0[:, gi * P:(gi + 1) * P],
                                       rhs=w1a[:], start=True, stop=False)
                      nc.tensor.matmul(out=pm[:],
                                       lhsT=xt1[:, gi * P:(gi + 1) * P],
                                       rhs=w1b[:], start=False, stop=True)
                      nc.vector.tensor_scalar_mul(
                          hs1c[:, gi * HID:(gi + 1) * HID], pm[:],
                          dvs[:, g:g + 1])
                  dst = hs1_own[a * P:b * P, :].rearrange(
                      "(c p) h -> p c h", p=P)
                  srcap = hs1c[:, 0:nb * HID].rearrange(
                      "p (c h) -> p c h", h=HID)
                  nc.sync.dma_start(out=dst, in_=srcap)

              nc.gpsimd.collective_compute(
                  "AllGather", mybir.AluOpType.bypass, replica_groups=rg,
                  ins=[hs1_own[:, :]], outs=[table1[:, :]])

              # ---- A1 + L2 projection ----
              h1c = None
              for L in layout:
                  a, b = L["a"], L["b"]
                  gt = gp_pool.tile([P, WMAXCH, HID], bf16, tag="g1")
                  ix = ixp.tile([P, MMAXCH], i16, tag="ix1")
                  nc.sync.dma_start(
                      out=ix[:, 0:sum(L["m"])],
                      in_=idx_t[:, L["mstart"]:L["mstart"] + sum(L["m"])])
                  cacc = 0
                  macc = 0
                  for s in range(NCLS):
                      cc = L["ccols"][s]
                      if cc == 0:
                          continue
                      emit_gather(gt[:, cacc:cacc + cc, :],
                                  tab1v[:, s, :],
                                  ix[:, macc:macc + L["m"][s]],
                                  128 * cc, HID)
                      cacc += cc
                      macc += L["m"][s]
                  for g in range(a, b):
                      parts = []
                      for s in range(NCLS):
                          w = int(Wgs[g, s])
                          if w == 0:
                              continue
                          off = L["offs"][(g, s)]
                          pr = wp.tile([P, HID], f32, tag=f"pr1_{s}")
                          nc.vector.tensor_reduce(
                              out=pr[:],
                              in_=gt[:, off:off + w, :].transpose([0, 2, 1]),
                              axis=AX, op=add)
                          parts.append(pr)
                      while len(parts) > 1:
                          nxt = []
                          for i in range(0, len(parts) - 1, 2):
                              t = wp.tile([P, HID], f32,
                                          tag=f"cmb1_{len(parts)}_{i}")
                              nc.vector.tensor_tensor(
                                  out=t[:], in0=parts[i][:],
                                  in1=parts[i + 1][:], op=add)
                              nxt.append(t)
                          if len(parts) % 2:
                              nxt.append(parts[-1])
                          parts = nxt
                      u = wp.tile([P, HID], f32, tag="u1")
                      nc.vector.tensor_scalar_mul(u[:], parts[0][:],
                                                  dvs[:, g:g + 1])
                      v = wp.tile([P, HID], f32, tag="v1")
                      nc.vector.tensor_tensor(out=v[:], in0=u[:], in1=b1s[:],
                                              op=add)
                      ti = g % TCH
                      if ti == 0:
                          h1c = h1p.tile([P, TCH * HID], bf16, tag="h1c")
                      nc.scalar.activation(
                          out=h1c[:, ti * HID:(ti + 1) * HID], in_=v[:],
                          func=relu, scale=dvs[:, g:g + 1])
                      if ti == TCH - 1 or g == NGROUP - 1:
                          g0 = g - ti
                          nbt = ti + 1
                          X = nbt * HID
                          pT = psT.tile([P, P], bf16, tag="pT")
                          nc.tensor.transpose(out=pT[0:X, :],
                                              in_=h1c[:, 0:X],
                                              identity=ident[:])
                          h1T = wp.tile([P, P], bf16, tag="h1T")
                          nc.vector.tensor_copy(out=h1T[0:X, :],
                                                in_=pT[0:X, :])
                          pm2 = ps2.tile([P, TCH * OUT_C], f32, tag="pm2")
                          nc.tensor.matmul(
                              out=pm2[:, 0:nbt * OUT_C],
                              lhsT=h1T[0:X, :],
                              rhs=w2blk[0:X, 0:nbt * OUT_C],
                              start=True, stop=True)
                          hs2c = wp.tile([P, TCH * OUT_C], bf16, tag="hs2c")
                          nc.vector.tensor_copy(out=hs2c[:, 0:nbt * OUT_C],
                                                in_=pm2[:, 0:nbt * OUT_C])
                          dstap = hs2_own[g0 * P:(g + 1) * P, 0:OUT_C] \
                              .rearrange("(c p) h -> p c h", p=P)
                          srcap = hs2c[:, 0:nbt * OUT_C].rearrange(
                              "p (c h) -> p c h", h=OUT_C)
                          nc.sync.dma_start(out=dstap, in_=srcap)

              nc.gpsimd.collective_compute(
                  "AllGather", mybir.AluOpType.bypass, replica_groups=rg,
                  ins=[hs2_own[:, :]], outs=[table2[:, :]])

              # ---- A2: final aggregation ----
              for L in layout:
                  a, b = L["a"], L["b"]
                  gt2 = gp_pool.tile([P, WMAXCH, OUT_C], bf16, tag="g2")
                  ix = ixp.tile([P, MMAXCH], i16, tag="ix2")
                  nc.sync.dma_start(
                      out=ix[:, 0:sum(L["m"])],
                      in_=idx_t[:, L["mstart"]:L["mstart"] + sum(L["m"])])
                  cacc = 0
                  macc = 0
                  for s in range(NCLS):
                      cc = L["ccols"][s]
                      if cc == 0:
                          continue
                      emit_gather(gt2[:, cacc:cacc + cc, :],
                                  tab2v[:, s, 0:OUT_C],
                                  ix[:, macc:macc + L["m"][s]],
                                  128 * cc, OUT_C)
                      cacc += cc
                      macc += L["m"][s]
                  o2c = wp.tile([P, 8 * OUT_C], f32, tag="o2c")
                  for g in range(a, b):
                      parts = []
                      for s in range(NCLS):
                          w = int(Wgs[g, s])
                          if w == 0:
                              continue
                          off = L["offs"][(g, s)]
                          pr = wp.tile([P, OUT_C], f32, tag=f"pr2_{s}")
                          nc.vector.tensor_reduce(
                              out=pr[:],
                              in_=gt2[:, off:off + w, :].transpose([0, 2, 1]),
                              axis=AX, op=add)
                          parts.append(pr)
                      while len(parts) > 1:
                          nxt = []
                          for i in range(0, len(parts) - 1, 2):
                              t = wp.tile([P, OUT_C], f32,
                                          tag=f"cmb2_{len(parts)}_{i}")
                              nc.vector.tensor_tensor(
                                  out=t[:], in0=parts[i][:],
                                  in1=parts[i + 1][:], op=add)
                              nxt.append(t)
                          if len(parts) % 2:
                              nxt.append(parts[-1])
                          parts = nxt
                      u2 = wp.tile([P, OUT_C], f32, tag="u2")
                      nc.vector.tensor_scalar_mul(u2[:], parts[0][:],
                                                  dvs[:, g:g + 1])
                      nc.vector.tensor_tensor(
                          out=o2c[:, (g - a) * OUT_C:(g - a + 1) * OUT_C],
                          in0=u2[:], in1=b2s[:], op=add)
                  dstap = out_t[a * P:b * P, :].rearrange(
                      "(c p) h -> p c h", p=P)
                  srcap = o2c[:, 0:(b - a) * OUT_C].rearrange(
                      "p (c h) -> p c h", h=OUT_C)
                  nc.sync.dma_start(out=dstap, in_=srcap)

    nc.compile()
    _NC_CACHE[key] = nc
    return nc


def kernel(x, edge_index, W1, b1, W2, b2):
    from concourse.bass_utils import run_bass_kernel_spmd

    prep = _host_prep(x, edge_index, W1, b1, W2, b2)
    nc = _build_bass(prep["build"])

    in_maps = []
    for c in range(NCORES):
        in_maps.append({
            "xT": prep["xT"][c],
            "idx": prep["idx"][c],
            "dinv": prep["dinv"][c],
            "W1": prep["W1"],
            "W2": prep["W2"],
            "b1b": prep["b1b"],
            "b2b": prep["b2b"],
        })
    import time as _time
    res = None
    for attempt in range(3):
        try:
            res = run_bass_kernel_spmd(nc, in_maps, core_ids=list(range(NCORES)))
            break
        except Exception:
            if attempt == 2:
                raise
            _time.sleep(15.0)
    assert res is not None

    out = np.empty((N, OUT_C), dtype=np.float32)
    for c in range(NCORES):
        out[prep["perms"][c]] = res.results[c]["out"][:NPC]
    return out


if __name__ == "__main__":
    rng = np.random.default_rng(0)
    x = rng.standard_normal((N, IN_C)).astype(np.float32)
    ei = rng.integers(0, N, size=(2, 3200000)).astype(np.int64)
    W1 = rng.standard_normal((IN_C, HID)).astype(np.float32) / 16.0
    W2 = rng.standard_normal((HID, OUT_C)).astype(np.float32) / 5.66
    out = kernel(x, ei, W1, np.zeros(HID, np.float32), W2,
                 np.zeros(OUT_C, np.float32))
    print(out.shape, out.dtype, np.abs(out).mean())


# revision 16
# speedup vs baseline: 1.5681x; 1.5681x over previous
"""GCN 2-layer encoder on 8 Trainium2 NeuronCores (Bass/Tile).

Math (PyG GCNConv x2, self-loops, symmetric norm):
    A' = A + I, deg = indegree(A'), dinv = deg^-1/2
    h1 = relu(dinv * (A' (dinv * (x W1))) + b1)
    out = dinv * (A' (dinv * (h1 W2))) + b2

Sharding: dst nodes split contiguously across 8 cores (12500 each). Each
core projects its own rows (x W), scales by dinv; an AllGather builds the
full projected table in DRAM; each core then aggregates its own dst rows
by gathering per-edge source rows (indirect DMA) and segment-summing with
DVE tensor_reduce over a degree-sorted, group-padded edge layout.

Host prep: nodes per core are sorted by degree and batched in groups of
128; group gather width D_g = max degree in the group (near-uniform after
sorting). The per-group widths are maxed across cores so all 8 cores run
one identical program (SPMD). Edge slots beyond a node's degree point at
an all-zeros table row.
"""

import numpy as np

N = 100000
IN_C, HID, OUT_C = 256, 32, 16
NCORES = 8
P = 128
NPC = N // NCORES            # nodes per core: 12500
NGROUP = (NPC + P - 1) // P  # 98 groups
NPAD = NGROUP * P            # 12544 rows per core slice (incl. dummies)
VTOT = NCORES * NPAD         # table rows: 100352


def _host_prep(x, edge_index, W1, b1, W2, b2):
    x = np.asarray(x, dtype=np.float32)
    ei = np.asarray(edge_index)
    W1 = np.asarray(W1, dtype=np.float32)
    b1 = np.asarray(b1, dtype=np.float32)
    W2 = np.asarray(W2, dtype=np.float32)
    b2 = np.asarray(b2, dtype=np.float32)

    loops = np.arange(N, dtype=np.int64)
    src = np.concatenate([ei[0], loops]).astype(np.int64)
    dst = np.concatenate([ei[1], loops]).astype(np.int64)

    deg = np.bincount(dst, minlength=N).astype(np.int64)
    dinv = (1.0 / np.sqrt(np.maximum(deg, 1))).astype(np.float32)

    core_of = (np.arange(N) // NPC).astype(np.int64)
    pos_in_core = np.empty(N, dtype=np.int64)
    perms = []
    for c in range(NCORES):
        nodes = np.arange(c * NPC, (c + 1) * NPC)
        perm = nodes[np.argsort(deg[nodes], kind="stable")]
        perms.append(perm)
        pos_in_core[perm] = np.arange(NPC)
    rowid = core_of * NPAD + pos_in_core  # table row of each node

    # per-(core, group) gather widths, maxed across cores for SPMD
    Dcg = np.zeros((NCORES, NGROUP), dtype=np.int64)
    for c in range(NCORES):
        dsort = deg[perms[c]]
        dpad = np.zeros(NPAD, dtype=np.int64)
        dpad[:NPC] = dsort
        Dcg[c] = dpad.reshape(NGROUP, P).max(axis=1)
    Dg = Dcg.max(axis=0)          # [NGROUP]
    Dg = np.maximum(Dg, 1)
    cumD = np.concatenate([[0], np.cumsum(Dg)]).astype(np.int64)
    sumD = int(cumD[-1])

    # CSR over table-row ids, then slot layout [core][p, cumD[g]+j]
    erow = rowid[dst]                              # dst slot row
    esrc_row = rowid[src].astype(np.int32)          # value to gather
    order = np.argsort(erow, kind="stable")
    erow_s = erow[order]
    esrc_s = esrc_row[order]
    counts = np.bincount(erow, minlength=VTOT)
    ptr = np.concatenate([[0], np.cumsum(counts)])
    j_idx = np.arange(erow_s.size, dtype=np.int64) - ptr[erow_s]

    c_arr = erow_s // NPAD
    within = erow_s % NPAD
    g_arr = within // P
    p_arr = within % P
    col_arr = cumD[g_arr] + j_idx

    zero_row = np.array([c * NPAD + NPC for c in range(NCORES)], dtype=np.int32)
    offs = np.empty((NCORES, P, sumD), dtype=np.int32)
    for c in range(NCORES):
        offs[c, :, :] = zero_row[c]
    offs[c_arr, p_arr, col_arr] = esrc_s

    # per-core inputs
    xT_list, dinv_list = [], []
    for c in range(NCORES):
        xp = np.zeros((NPAD, IN_C), dtype=np.float32)
        xp[:NPC] = x[perms[c]]
        xT_list.append(np.ascontiguousarray(xp.T))
        dv = np.zeros(NPAD, dtype=np.float32)
        dv[:NPC] = dinv[perms[c]]
        dinv_list.append(np.ascontiguousarray(
            dv.reshape(NGROUP, P).T))  # [128, NGROUP]

    wrows_list = []
    for c in range(NCORES):
        base = c * NPAD
        wr = (base + np.arange(NPAD, dtype=np.int32)).reshape(NGROUP, P).T
        wrows_list.append(np.ascontiguousarray(wr))  # [128, NGROUP]

    b1b = np.tile(b1[None, :], (P, 1)).astype(np.float32)
    b2b = np.tile(b2[None, :], (P, 1)).astype(np.float32)

    return dict(
        Dg=Dg, cumD=cumD, sumD=sumD, offs=offs, xT=xT_list, dinv=dinv_list,
        W1=W1, W2=W2, b1b=b1b, b2b=b2b, perms=perms, wrows=wrows_list,
    )


_NC_CACHE = {}


def _build_bass(Dg, sumD, repeat=1, xcc=1):
    key = (tuple(int(d) for d in Dg), int(sumD), repeat, xcc)
    if key in _NC_CACHE:
        return _NC_CACHE[key]

    import concourse.bacc as bacc
    import concourse.bass as bass
    import concourse.tile as tile
    import concourse.mybir as mybir
    from concourse.masks import make_identity
    from concourse.tile import add_dep_helper

    f32 = mybir.dt.float32
    i32 = mybir.dt.int32
    cumD = np.concatenate([[0], np.cumsum(Dg)]).astype(np.int64)

    nc = bacc.Bacc("TRN2", target_bir_lowering=False, debug=False,
                   num_devices=NCORES)

    xT_t = nc.dram_tensor("xT", [IN_C, NPAD], f32, kind="ExternalInput")
    offs_t = nc.dram_tensor("offs", [P, sumD], i32, kind="ExternalInput")
    dinv_t = nc.dram_tensor("dinv", [P, NGROUP], f32, kind="ExternalInput")
    W1_t = nc.dram_tensor("W1", [IN_C, HID], f32, kind="ExternalInput")
    W2_t = nc.dram_tensor("W2", [HID, OUT_C], f32, kind="ExternalInput")
    b1b_t = nc.dram_tensor("b1b", [P, HID], f32, kind="ExternalInput")
    b2b_t = nc.dram_tensor("b2b", [P, OUT_C], f32, kind="ExternalInput")
    out_t = nc.dram_tensor("out", [NPAD, OUT_C], f32, kind="ExternalOutput")

    hs1_own = nc.dram_tensor("hs1_own", [NPAD, HID], f32)
    hs2_own = nc.dram_tensor("hs2_own", [NPAD, OUT_C], f32)
    table1 = nc.dram_tensor("table1", [VTOT, HID], f32, addr_space="Shared")
    table2 = nc.dram_tensor("table2", [VTOT, OUT_C], f32, addr_space="Shared")

    groups = list(range(NGROUP))
    rg = [list(range(NCORES))]

    with tile.TileContext(nc) as tc:
        with tc.tile_pool(name="const", bufs=1) as cp, \
             tc.tile_pool(name="xt", bufs=4) as xp, \
             tc.tile_pool(name="gat", bufs=8) as gp, \
             tc.tile_pool(name="work", bufs=4) as wp, \
             tc.tile_pool(name="ps1", bufs=2, space="PSUM") as ps1, \
             tc.tile_pool(name="psT", bufs=2, space="PSUM") as psT, \
             tc.tile_pool(name="ps2", bufs=2, space="PSUM") as ps2:

            ident = cp.tile([P, P], f32)
            make_identity(nc, ident[:])
            w1a = cp.tile([P, HID], f32)
            w1b = cp.tile([P, HID], f32)
            nc.sync.dma_start(out=w1a[:], in_=W1_t[0:P, :])
            nc.sync.dma_start(out=w1b[:], in_=W1_t[P:IN_C, :])
            w2s = cp.tile([HID, OUT_C], f32)
            nc.sync.dma_start(out=w2s[:], in_=W2_t[:, :])
            b1s = cp.tile([P, HID], f32)
            nc.sync.dma_start(out=b1s[:], in_=b1b_t[:, :])
            b2s = cp.tile([P, OUT_C], f32)
            nc.sync.dma_start(out=b2s[:], in_=b2b_t[:, :])
            dvs = cp.tile([P, NGROUP], f32)
            nc.sync.dma_start(out=dvs[:], in_=dinv_t[:, :])
            offs_sb = cp.tile([P, sumD], i32)
            nc.sync.dma_start(out=offs_sb[:], in_=offs_t[:, :])

            # ---- P1: project own rows, scatter into shared table1 ----
            for _rep in range(repeat):
              for g in groups:
                  xt0 = xp.tile([P, P], f32, tag="xt0")
                  xt1 = xp.tile([P, P], f32, tag="xt1")
                  nc.sync.dma_start(out=xt0[:], in_=xT_t[0:P, g * P:(g + 1) * P])
                  nc.sync.dma_start(out=xt1[:], in_=xT_t[P:IN_C, g * P:(g + 1) * P])
                  pm = ps1.tile([P, HID], f32)
                  nc.tensor.matmul(out=pm[:], lhsT=xt0[:], rhs=w1a[:],
                                   start=True, stop=False)
                  nc.tensor.matmul(out=pm[:], lhsT=xt1[:], rhs=w1b[:],
                                   start=False, stop=True)
                  hs1 = wp.tile([P, HID], f32, tag="hs1")
                  nc.vector.tensor_scalar_mul(hs1[:], pm[:], dvs[:, g:g + 1])
                  nc.sync.dma_start(out=hs1_own[g * P:(g + 1) * P, :], in_=hs1[:])

              nc.gpsimd.collective_compute(
                  "AllGather", mybir.AluOpType.bypass, replica_groups=rg,
                  ins=[hs1_own[:, :]], outs=[table1[:, :]])

              # ---- A1 + L2 projection, per group ----
              NACC = 4
              for g in groups:
                  D = int(Dg[g])
                  nacc = min(NACC, D)
                  accs = [gp.tile([P, HID], f32, tag=f"a1_{k}", name=f"a1g{g}_{k}") for k in range(nacc)]
                  for j in range(D):
                      col = int(cumD[g]) + j
                      gi = nc.gpsimd.indirect_dma_start(
                          out=accs[j % nacc][:],
                          out_offset=None,
                          in_=table1[:, :],
                          in_offset=bass.IndirectOffsetOnAxis(
                              ap=offs_sb[:, col:col + 1], axis=0),
                          compute_op=(mybir.AluOpType.bypass if j < nacc
                                      else mybir.AluOpType.add),
                      )
                  red = wp.tile([P, HID], f32, tag="red1")
                  if nacc == 1:
                      nc.vector.tensor_copy(out=red[:], in_=accs[0][:])
                  else:
                      while len(accs) > 2:
                          t = wp.tile([P, HID], f32, tag="cmb1")
                          nc.vector.tensor_tensor(out=t[:], in0=accs[0][:],
                                                  in1=accs[1][:],
                                                  op=mybir.AluOpType.add)
                          accs = [t] + accs[2:]
                      nc.vector.tensor_tensor(out=red[:], in0=accs[0][:],
                                              in1=accs[1][:],
                                              op=mybir.AluOpType.add)
                  u = wp.tile([P, HID], f32, tag="u1")
                  nc.vector.tensor_scalar_mul(u[:], red[:], dvs[:, g:g + 1])
                  v = wp.tile([P, HID], f32, tag="v1")
                  nc.vector.tensor_tensor(out=v[:], in0=u[:], in1=b1s[:],
                                          op=mybir.AluOpType.add)
                  h1 = wp.tile([P, HID], f32, tag="h1")
                  nc.scalar.activation(out=h1[:], in_=v[:],
                                       func=mybir.ActivationFunctionType.Relu,
                                       scale=dvs[:, g:g + 1])
                  # L2 projection: hs2 = (dinv*h1) @ W2  (dinv already folded)
                  pT = psT.tile([HID, P], f32)
                  nc.tensor.transpose(out=pT[:], in_=h1[:], identity=ident[:])
                  h1T = wp.tile([HID, P], f32, tag="h1T")
                  nc.vector.tensor_copy(out=h1T[:], in_=pT[:])
                  pm2 = ps2.tile([P, OUT_C], f32)
                  nc.tensor.matmul(out=pm2[:], lhsT=h1T[:], rhs=w2s[:],
                                   start=True, stop=True)
                  hs2 = wp.tile([P, OUT_C], f32, tag="hs2")
                  nc.vector.tensor_copy(out=hs2[:], in_=pm2[:])
                  nc.sync.dma_start(out=hs2_own[g * P:(g + 1) * P, :], in_=hs2[:])

              nc.gpsimd.collective_compute(
                  "AllGather", mybir.AluOpType.bypass, replica_groups=rg,
                  ins=[hs2_own[:, :]], outs=[table2[:, :]])

              # ---- A2: final aggregation ----
              for g in groups:
                  D = int(Dg[g])
                  nacc = min(NACC, D)
                  accs = [gp.tile([P, OUT_C], f32, tag=f"a2_{k}", name=f"a2g{g}_{k}") for k in range(nacc)]
                  for j in range(D):
                      col = int(cumD[g]) + j
                      gi2 = nc.gpsimd.indirect_dma_start(
                          out=accs[j % nacc][:],
                          out_offset=None,
                          in_=table2[:, :],
                          in_offset=bass.IndirectOffsetOnAxis(
                              ap=offs_sb[:, col:col + 1], axis=0),
                          compute_op=(mybir.AluOpType.bypass if j < nacc
                                      else mybir.AluOpType.add),
                      )
                  red2 = wp.tile([P, OUT_C], f32, tag="red2")
                  if nacc == 1:
                      nc.vector.tensor_copy(out=red2[:], in_=accs[0][:])
                  else:
                      while len(accs) > 2:
                          t = wp.tile([P, OUT_C], f32, tag="cmb2")
                          nc.vector.tensor_tensor(out=t[:], in0=accs[0][:],
                                                  in1=accs[1][:],
                                                  op=mybir.AluOpType.add)
                          accs = [t] + accs[2:]
                      nc.vector.tensor_tensor(out=red2[:], in0=accs[0][:],
                                              in1=accs[1][:],
                                              op=mybir.AluOpType.add)
                  u2 = wp.tile([P, OUT_C], f32, tag="u2")
                  nc.vector.tensor_scalar_mul(u2[:], red2[:], dvs[:, g:g + 1])
                  o2 = wp.tile([P, OUT_C], f32, tag="o2")
                  nc.vector.tensor_tensor(out=o2[:], in0=u2[:], in1=b2s[:],
                                          op=mybir.AluOpType.add)
                  nc.sync.dma_start(out=out_t[g * P:(g + 1) * P, :], in_=o2[:])

    nc.compile()
    _NC_CACHE[key] = nc
    return nc


def kernel(x, edge_index, W1, b1, W2, b2):
    from concourse.bass_utils import run_bass_kernel_spmd

    prep = _host_prep(x, edge_index, W1, b1, W2, b2)
    nc = _build_bass(prep["Dg"], prep["sumD"])

    in_maps = []
    for c in range(NCORES):
        in_maps.append({
            "xT": prep["xT"][c],
            "offs": np.ascontiguousarray(prep["offs"][c]),
            "dinv": prep["dinv"][c],
            "W1": prep["W1"],
            "W2": prep["W2"],
            "b1b": prep["b1b"],
            "b2b": prep["b2b"],
        })
    import time as _time
    res = None
    for attempt in range(3):
        try:
            res = run_bass_kernel_spmd(nc, in_maps, core_ids=list(range(NCORES)))
            break
        except Exception:
            if attempt == 2:
                raise
            _time.sleep(15.0)
    assert res is not None

    out = np.empty((N, OUT_C), dtype=np.float32)
    for c in range(NCORES):
        out[prep["perms"][c]] = res.results[c]["out"][:NPC]
    return out


if __name__ == "__main__":
    rng = np.random.default_rng(0)
    x = rng.standard_normal((N, IN_C)).astype(np.float32)
    ei = rng.integers(0, N, size=(2, 3200000)).astype(np.int64)
    W1 = rng.standard_normal((IN_C, HID)).astype(np.float32) / 16.0
    W2 = rng.standard_normal((HID, OUT_C)).astype(np.float32) / 5.66
    out = kernel(x, ei, W1, np.zeros(HID, np.float32), W2,
                 np.zeros(OUT_C, np.float32))
    print(out.shape, out.dtype, np.abs(out).mean())



# revision 19
# speedup vs baseline: 1.6420x; 1.0471x over previous
"""GCN 2-layer encoder on 8 Trainium2 NeuronCores (Bass/Tile).

Math (PyG GCNConv x2, self-loops, symmetric norm):
    A' = A + I, deg = indegree(A'), dinv = deg^-1/2
    h1 = relu(dinv * (A' (dinv * (x W1))) + b1)
    out = dinv * (A' (dinv * (h1 W2))) + b2

Sharding: dst nodes split contiguously across 8 cores (12500 each). Each
core projects its own rows (x W), scales by dinv; an AllGather builds the
full projected table in DRAM; each core then aggregates its own dst rows
by gathering per-edge source rows (indirect DMA) and segment-summing with
DVE tensor_reduce over a degree-sorted, group-padded edge layout.

Host prep: nodes per core are sorted by degree and batched in groups of
128; group gather width D_g = max degree in the group (near-uniform after
sorting). The per-group widths are maxed across cores so all 8 cores run
one identical program (SPMD). Edge slots beyond a node's degree point at
an all-zeros table row.
"""

import numpy as np

N = 100000
IN_C, HID, OUT_C = 256, 32, 16
NCORES = 8
P = 128
NPC = N // NCORES            # nodes per core: 12500
NGROUP = (NPC + P - 1) // P  # 98 groups
NPAD = NGROUP * P            # 12544 rows per core slice (incl. dummies)
VTOT = NCORES * NPAD         # table rows: 100352


def _host_prep(x, edge_index, W1, b1, W2, b2):
    x = np.asarray(x, dtype=np.float32)
    ei = np.asarray(edge_index)
    W1 = np.asarray(W1, dtype=np.float32)
    b1 = np.asarray(b1, dtype=np.float32)
    W2 = np.asarray(W2, dtype=np.float32)
    b2 = np.asarray(b2, dtype=np.float32)

    # self-loops are handled locally (h[v] added from hs*_own on-device),
    # so the gather layout only covers the real edges of A
    src = ei[0].astype(np.int64)
    dst = ei[1].astype(np.int64)

    deg_ns = np.bincount(dst, minlength=N).astype(np.int64)  # without +I
    deg = deg_ns + 1
    dinv = (1.0 / np.sqrt(deg)).astype(np.float32)

    core_of = (np.arange(N) // NPC).astype(np.int64)
    pos_in_core = np.empty(N, dtype=np.int64)
    perms = []
    for c in range(NCORES):
        nodes = np.arange(c * NPC, (c + 1) * NPC)
        perm = nodes[np.argsort(deg_ns[nodes], kind="stable")]
        perms.append(perm)
        pos_in_core[perm] = np.arange(NPC)
    rowid = core_of * NPAD + pos_in_core  # table row of each node

    # per-(core, group) gather widths, maxed across cores for SPMD
    Dcg = np.zeros((NCORES, NGROUP), dtype=np.int64)
    for c in range(NCORES):
        dsort = deg_ns[perms[c]]
        dpad = np.zeros(NPAD, dtype=np.int64)
        dpad[:NPC] = dsort
        Dcg[c] = dpad.reshape(NGROUP, P).max(axis=1)
    Dg = Dcg.max(axis=0)          # [NGROUP]
    Dg = np.maximum(Dg, 1)
    cumD = np.concatenate([[0], np.cumsum(Dg)]).astype(np.int64)
    sumD = int(cumD[-1])

    # CSR over table-row ids, then slot layout [core][p, cumD[g]+j]
    erow = rowid[dst]                              # dst slot row
    esrc_row = rowid[src].astype(np.int32)          # value to gather
    order = np.argsort(erow, kind="stable")
    erow_s = erow[order]
    esrc_s = esrc_row[order]
    counts = np.bincount(erow, minlength=VTOT)
    ptr = np.concatenate([[0], np.cumsum(counts)])
    j_idx = np.arange(erow_s.size, dtype=np.int64) - ptr[erow_s]

    c_arr = erow_s // NPAD
    within = erow_s % NPAD
    g_arr = within // P
    p_arr = within % P
    col_arr = cumD[g_arr] + j_idx

    zero_row = np.array([c * NPAD + NPC for c in range(NCORES)], dtype=np.int32)
    offs = np.empty((NCORES, P, sumD), dtype=np.int32)
    for c in range(NCORES):
        offs[c, :, :] = zero_row[c]
    offs[c_arr, p_arr, col_arr] = esrc_s

    # per-core inputs
    xT_list, dinv_list = [], []
    for c in range(NCORES):
        xp = np.zeros((NPAD, IN_C), dtype=np.float32)
        xp[:NPC] = x[perms[c]]
        xT_list.append(np.ascontiguousarray(xp.T))
        dv = np.zeros(NPAD, dtype=np.float32)
        dv[:NPC] = dinv[perms[c]]
        dinv_list.append(np.ascontiguousarray(
            dv.reshape(NGROUP, P).T))  # [128, NGROUP]

    wrows_list = []
    for c in range(NCORES):
        base = c * NPAD
        wr = (base + np.arange(NPAD, dtype=np.int32)).reshape(NGROUP, P).T
        wrows_list.append(np.ascontiguousarray(wr))  # [128, NGROUP]

    b1b = np.tile(b1[None, :], (P, 1)).astype(np.float32)
    b2b = np.tile(b2[None, :], (P, 1)).astype(np.float32)

    return dict(
        Dg=Dg, cumD=cumD, sumD=sumD, offs=offs, xT=xT_list, dinv=dinv_list,
        W1=W1, W2=W2, b1b=b1b, b2b=b2b, perms=perms, wrows=wrows_list,
    )


_NC_CACHE = {}


def _build_bass(Dg, sumD, repeat=1, xcc=1):
    key = (tuple(int(d) for d in Dg), int(sumD), repeat, xcc)
    if key in _NC_CACHE:
        return _NC_CACHE[key]

    import concourse.bacc as bacc
    import concourse.bass as bass
    import concourse.tile as tile
    import concourse.mybir as mybir
    from concourse.masks import make_identity
    from concourse.tile import add_dep_helper

    f32 = mybir.dt.float32
    i32 = mybir.dt.int32
    cumD = np.concatenate([[0], np.cumsum(Dg)]).astype(np.int64)

    nc = bacc.Bacc("TRN2", target_bir_lowering=False, debug=False,
                   num_devices=NCORES)

    xT_t = nc.dram_tensor("xT", [IN_C, NPAD], f32, kind="ExternalInput")
    offs_t = nc.dram_tensor("offs", [P, sumD], i32, kind="ExternalInput")
    dinv_t = nc.dram_tensor("dinv", [P, NGROUP], f32, kind="ExternalInput")
    W1_t = nc.dram_tensor("W1", [IN_C, HID], f32, kind="ExternalInput")
    W2_t = nc.dram_tensor("W2", [HID, OUT_C], f32, kind="ExternalInput")
    b1b_t = nc.dram_tensor("b1b", [P, HID], f32, kind="ExternalInput")
    b2b_t = nc.dram_tensor("b2b", [P, OUT_C], f32, kind="ExternalInput")
    out_t = nc.dram_tensor("out", [NPAD, OUT_C], f32, kind="ExternalOutput")

    hs1_own = nc.dram_tensor("hs1_own", [NPAD, HID], f32)
    hs2_own = nc.dram_tensor("hs2_own", [NPAD, OUT_C], f32)
    table1 = nc.dram_tensor("table1", [VTOT, HID], f32, addr_space="Shared")
    table2 = nc.dram_tensor("table2", [VTOT, OUT_C], f32, addr_space="Shared")

    groups = list(range(NGROUP))
    rg = [list(range(NCORES))]

    with tile.TileContext(nc) as tc:
        with tc.tile_pool(name="const", bufs=1) as cp, \
             tc.tile_pool(name="xt", bufs=4) as xp, \
             tc.tile_pool(name="gat", bufs=8) as gp, \
             tc.tile_pool(name="work", bufs=4) as wp, \
             tc.tile_pool(name="ps1", bufs=2, space="PSUM") as ps1, \
             tc.tile_pool(name="psT", bufs=2, space="PSUM") as psT, \
             tc.tile_pool(name="ps2", bufs=2, space="PSUM") as ps2:

            ident = cp.tile([P, P], f32)
            make_identity(nc, ident[:])
            w1a = cp.tile([P, HID], f32)
            w1b = cp.tile([P, HID], f32)
            nc.sync.dma_start(out=w1a[:], in_=W1_t[0:P, :])
            nc.sync.dma_start(out=w1b[:], in_=W1_t[P:IN_C, :])
            w2s = cp.tile([HID, OUT_C], f32)
            nc.sync.dma_start(out=w2s[:], in_=W2_t[:, :])
            b1s = cp.tile([P, HID], f32)
            nc.sync.dma_start(out=b1s[:], in_=b1b_t[:, :])
            b2s = cp.tile([P, OUT_C], f32)
            nc.sync.dma_start(out=b2s[:], in_=b2b_t[:, :])
            dvs = cp.tile([P, NGROUP], f32)
            nc.sync.dma_start(out=dvs[:], in_=dinv_t[:, :])
            offs_sb = cp.tile([P, sumD], i32)
            nc.sync.dma_start(out=offs_sb[:], in_=offs_t[:, :])

            # ---- P1: project own rows, scatter into shared table1 ----
            for _rep in range(repeat):
              for g in groups:
                  xt0 = xp.tile([P, P], f32, tag="xt0")
                  xt1 = xp.tile([P, P], f32, tag="xt1")
                  nc.sync.dma_start(out=xt0[:], in_=xT_t[0:P, g * P:(g + 1) * P])
                  nc.sync.dma_start(out=xt1[:], in_=xT_t[P:IN_C, g * P:(g + 1) * P])
                  pm = ps1.tile([P, HID], f32)
                  nc.tensor.matmul(out=pm[:], lhsT=xt0[:], rhs=w1a[:],
                                   start=True, stop=False)
                  nc.tensor.matmul(out=pm[:], lhsT=xt1[:], rhs=w1b[:],
                                   start=False, stop=True)
                  hs1 = wp.tile([P, HID], f32, tag="hs1")
                  nc.vector.tensor_scalar_mul(hs1[:], pm[:], dvs[:, g:g + 1])
                  nc.sync.dma_start(out=hs1_own[g * P:(g + 1) * P, :], in_=hs1[:])

              nc.gpsimd.collective_compute(
                  "AllGather", mybir.AluOpType.bypass, replica_groups=rg,
                  ins=[hs1_own[:, :]], outs=[table1[:, :]])

              # ---- A1 + L2 projection, per group ----
              NACC = 4
              for g in groups:
                  D = int(Dg[g])
                  nacc = min(NACC, D)
                  # self-loop term: own hs1 rows, no gather needed
                  sf1 = wp.tile([P, HID], f32, tag="sf1")
                  nc.sync.dma_start(out=sf1[:],
                                    in_=hs1_own[g * P:(g + 1) * P, :])
                  accs = [gp.tile([P, HID], f32, tag=f"a1_{k}", name=f"a1g{g}_{k}") for k in range(nacc)]
                  for j in range(D):
                      col = int(cumD[g]) + j
                      gi = nc.gpsimd.indirect_dma_start(
                          out=accs[j % nacc][:],
                          out_offset=None,
                          in_=table1[:, :],
                          in_offset=bass.IndirectOffsetOnAxis(
                              ap=offs_sb[:, col:col + 1], axis=0),
                          compute_op=(mybir.AluOpType.bypass if j < nacc
                                      else mybir.AluOpType.add),
                      )
                  accs.append(sf1)
                  red = wp.tile([P, HID], f32, tag="red1")
                  while len(accs) > 2:
                      t = wp.tile([P, HID], f32, tag="cmb1")
                      nc.vector.tensor_tensor(out=t[:], in0=accs[0][:],
                                              in1=accs[1][:],
                                              op=mybir.AluOpType.add)
                      accs = [t] + accs[2:]
                  nc.vector.tensor_tensor(out=red[:], in0=accs[0][:],
                                          in1=accs[1][:],
                                          op=mybir.AluOpType.add)
                  u = wp.tile([P, HID], f32, tag="u1")
                  nc.vector.tensor_scalar_mul(u[:], red[:], dvs[:, g:g + 1])
                  v = wp.tile([P, HID], f32, tag="v1")
                  nc.vector.tensor_tensor(out=v[:], in0=u[:], in1=b1s[:],
                                          op=mybir.AluOpType.add)
                  h1 = wp.tile([P, HID], f32, tag="h1")
                  nc.scalar.activation(out=h1[:], in_=v[:],
                                       func=mybir.ActivationFunctionType.Relu,
                                       scale=dvs[:, g:g + 1])
                  # L2 projection: hs2 = (dinv*h1) @ W2  (dinv already folded)
                  pT = psT.tile([HID, P], f32)
                  nc.tensor.transpose(out=pT[:], in_=h1[:], identity=ident[:])
                  h1T = wp.tile([HID, P], f32, tag="h1T")
                  nc.vector.tensor_copy(out=h1T[:], in_=pT[:])
                  pm2 = ps2.tile([P, OUT_C], f32)
                  nc.tensor.matmul(out=pm2[:], lhsT=h1T[:], rhs=w2s[:],
                                   start=True, stop=True)
                  hs2 = wp.tile([P, OUT_C], f32, tag="hs2")
                  nc.vector.tensor_copy(out=hs2[:], in_=pm2[:])
                  nc.sync.dma_start(out=hs2_own[g * P:(g + 1) * P, :], in_=hs2[:])

              nc.gpsimd.collective_compute(
                  "AllGather", mybir.AluOpType.bypass, replica_groups=rg,
                  ins=[hs2_own[:, :]], outs=[table2[:, :]])

              # ---- A2: final aggregation ----
              for g in groups:
                  D = int(Dg[g])
                  nacc = min(NACC, D)
                  sf2 = wp.tile([P, OUT_C], f32, tag="sf2")
                  nc.sync.dma_start(out=sf2[:],
                                    in_=hs2_own[g * P:(g + 1) * P, :])
                  accs = [gp.tile([P, OUT_C], f32, tag=f"a2_{k}", name=f"a2g{g}_{k}") for k in range(nacc)]
                  for j in range(D):
                      col = int(cumD[g]) + j
                      gi2 = nc.gpsimd.indirect_dma_start(
                          out=accs[j % nacc][:],
                          out_offset=None,
                          in_=table2[:, :],
                          in_offset=bass.IndirectOffsetOnAxis(
                              ap=offs_sb[:, col:col + 1], axis=0),
                          compute_op=(mybir.AluOpType.bypass if j < nacc
                                      else mybir.AluOpType.add),
                      )
                  accs.append(sf2)
                  red2 = wp.tile([P, OUT_C], f32, tag="red2")
                  while len(accs) > 2:
                      t = wp.tile([P, OUT_C], f32, tag="cmb2")
                      nc.vector.tensor_tensor(out=t[:], in0=accs[0][:],
                                              in1=accs[1][:],
                                              op=mybir.AluOpType.add)
                      accs = [t] + accs[2:]
                  nc.vector.tensor_tensor(out=red2[:], in0=accs[0][:],
                                          in1=accs[1][:],
                                          op=mybir.AluOpType.add)
                  u2 = wp.tile([P, OUT_C], f32, tag="u2")
                  nc.vector.tensor_scalar_mul(u2[:], red2[:], dvs[:, g:g + 1])
                  o2 = wp.tile([P, OUT_C], f32, tag="o2")
                  nc.vector.tensor_tensor(out=o2[:], in0=u2[:], in1=b2s[:],
                                          op=mybir.AluOpType.add)
                  nc.sync.dma_start(out=out_t[g * P:(g + 1) * P, :], in_=o2[:])

    nc.compile()
    _NC_CACHE[key] = nc
    return nc


def kernel(x, edge_index, W1, b1, W2, b2):
    from concourse.bass_utils import run_bass_kernel_spmd

    prep = _host_prep(x, edge_index, W1, b1, W2, b2)
    nc = _build_bass(prep["Dg"], prep["sumD"])

    in_maps = []
    for c in range(NCORES):
        in_maps.append({
            "xT": prep["xT"][c],
            "offs": np.ascontiguousarray(prep["offs"][c]),
            "dinv": prep["dinv"][c],
            "W1": prep["W1"],
            "W2": prep["W2"],
            "b1b": prep["b1b"],
            "b2b": prep["b2b"],
        })
    import time as _time
    res = None
    for attempt in range(3):
        try:
            res = run_bass_kernel_spmd(nc, in_maps, core_ids=list(range(NCORES)))
            break
        except Exception:
            if attempt == 2:
                raise
            _time.sleep(15.0)
    assert res is not None

    out = np.empty((N, OUT_C), dtype=np.float32)
    for c in range(NCORES):
        out[prep["perms"][c]] = res.results[c]["out"][:NPC]
    return out


if __name__ == "__main__":
    rng = np.random.default_rng(0)
    x = rng.standard_normal((N, IN_C)).astype(np.float32)
    ei = rng.integers(0, N, size=(2, 3200000)).astype(np.int64)
    W1 = rng.standard_normal((IN_C, HID)).astype(np.float32) / 16.0
    W2 = rng.standard_normal((HID, OUT_C)).astype(np.float32) / 5.66
    out = kernel(x, ei, W1, np.zeros(HID, np.float32), W2,
                 np.zeros(OUT_C, np.float32))
    print(out.shape, out.dtype, np.abs(out).mean())



# revision 21
# speedup vs baseline: 1.6741x; 1.0196x over previous
"""GCN 2-layer encoder on 8 Trainium2 NeuronCores (Bass/Tile).

Math (PyG GCNConv x2, self-loops, symmetric norm):
    A' = A + I, deg = indegree(A'), dinv = deg^-1/2
    h1 = relu(dinv * (A' (dinv * (x W1))) + b1)
    out = dinv * (A' (dinv * (h1 W2))) + b2

Sharding: dst nodes split contiguously across 8 cores (12500 each). Each
core projects its own rows (x W), scales by dinv; an AllGather builds the
full projected table in DRAM; each core then aggregates its own dst rows
by gathering per-edge source rows (indirect DMA) and segment-summing with
DVE tensor_reduce over a degree-sorted, group-padded edge layout.

Host prep: nodes per core are sorted by degree and batched in groups of
128; group gather width D_g = max degree in the group (near-uniform after
sorting). The per-group widths are maxed across cores so all 8 cores run
one identical program (SPMD). Edge slots beyond a node's degree point at
an all-zeros table row.
"""

import numpy as np

N = 100000
IN_C, HID, OUT_C = 256, 32, 16
NCORES = 8
P = 128
NPC = N // NCORES            # nodes per core: 12500
NGROUP = (NPC + P - 1) // P  # 98 groups
NPAD = NGROUP * P            # 12544 rows per core slice (incl. dummies)
VTOT = NCORES * NPAD         # table rows: 100352


def _host_prep(x, edge_index, W1, b1, W2, b2):
    x = np.asarray(x, dtype=np.float32)
    ei = np.asarray(edge_index)
    W1 = np.asarray(W1, dtype=np.float32)
    b1 = np.asarray(b1, dtype=np.float32)
    W2 = np.asarray(W2, dtype=np.float32)
    b2 = np.asarray(b2, dtype=np.float32)

    # self-loops are handled locally (h[v] added from hs*_own on-device),
    # so the gather layout only covers the real edges of A
    src = ei[0].astype(np.int64)
    dst = ei[1].astype(np.int64)

    deg_ns = np.bincount(dst, minlength=N).astype(np.int64)  # without +I
    deg = deg_ns + 1
    dinv = (1.0 / np.sqrt(deg)).astype(np.float32)

    core_of = (np.arange(N) // NPC).astype(np.int64)
    pos_in_core = np.empty(N, dtype=np.int64)
    perms = []
    for c in range(NCORES):
        nodes = np.arange(c * NPC, (c + 1) * NPC)
        perm = nodes[np.argsort(deg_ns[nodes], kind="stable")]
        perms.append(perm)
        pos_in_core[perm] = np.arange(NPC)
    rowid = core_of * NPAD + pos_in_core  # table row of each node

    # per-(core, group) gather widths, maxed across cores for SPMD
    Dcg = np.zeros((NCORES, NGROUP), dtype=np.int64)
    for c in range(NCORES):
        dsort = deg_ns[perms[c]]
        dpad = np.zeros(NPAD, dtype=np.int64)
        dpad[:NPC] = dsort
        Dcg[c] = dpad.reshape(NGROUP, P).max(axis=1)
    Dg = Dcg.max(axis=0)          # [NGROUP]
    Dg = np.maximum(Dg, 1)
    cumD = np.concatenate([[0], np.cumsum(Dg)]).astype(np.int64)
    sumD = int(cumD[-1])

    # CSR over table-row ids, then slot layout [core][p, cumD[g]+j]
    erow = rowid[dst]                              # dst slot row
    esrc_row = rowid[src].astype(np.int32)          # value to gather
    order = np.argsort(erow, kind="stable")
    erow_s = erow[order]
    esrc_s = esrc_row[order]
    counts = np.bincount(erow, minlength=VTOT)
    ptr = np.concatenate([[0], np.cumsum(counts)])
    j_idx = np.arange(erow_s.size, dtype=np.int64) - ptr[erow_s]

    c_arr = erow_s // NPAD
    within = erow_s % NPAD
    g_arr = within // P
    p_arr = within % P
    col_arr = cumD[g_arr] + j_idx

    zero_row = np.array([c * NPAD + NPC for c in range(NCORES)], dtype=np.int32)
    offs = np.empty((NCORES, P, sumD), dtype=np.int32)
    for c in range(NCORES):
        offs[c, :, :] = zero_row[c]
    offs[c_arr, p_arr, col_arr] = esrc_s

    # per-core inputs
    xT_list, dinv_list = [], []
    for c in range(NCORES):
        xp = np.zeros((NPAD, IN_C), dtype=np.float32)
        xp[:NPC] = x[perms[c]]
        xT_list.append(np.ascontiguousarray(xp.T))
        dv = np.zeros(NPAD, dtype=np.float32)
        dv[:NPC] = dinv[perms[c]]
        dinv_list.append(np.ascontiguousarray(
            dv.reshape(NGROUP, P).T))  # [128, NGROUP]

    wrows_list = []
    for c in range(NCORES):
        base = c * NPAD
        wr = (base + np.arange(NPAD, dtype=np.int32)).reshape(NGROUP, P).T
        wrows_list.append(np.ascontiguousarray(wr))  # [128, NGROUP]

    b1b = np.tile(b1[None, :], (P, 1)).astype(np.float32)
    b2b = np.tile(b2[None, :], (P, 1)).astype(np.float32)

    return dict(
        Dg=Dg, cumD=cumD, sumD=sumD, offs=offs, xT=xT_list, dinv=dinv_list,
        W1=W1, W2=W2, b1b=b1b, b2b=b2b, perms=perms, wrows=wrows_list,
    )


_NC_CACHE = {}


def _build_bass(Dg, sumD, repeat=1, xcc=1):
    key = (tuple(int(d) for d in Dg), int(sumD), repeat, xcc)
    if key in _NC_CACHE:
        return _NC_CACHE[key]

    import concourse.bacc as bacc
    import concourse.bass as bass
    import concourse.tile as tile
    import concourse.mybir as mybir
    from concourse.masks import make_identity
    from concourse.tile import add_dep_helper

    f32 = mybir.dt.float32
    i32 = mybir.dt.int32
    cumD = np.concatenate([[0], np.cumsum(Dg)]).astype(np.int64)

    nc = bacc.Bacc("TRN2", target_bir_lowering=False, debug=False,
                   num_devices=NCORES)

    xT_t = nc.dram_tensor("xT", [IN_C, NPAD], f32, kind="ExternalInput")
    offs_t = nc.dram_tensor("offs", [P, sumD], i32, kind="ExternalInput")
    dinv_t = nc.dram_tensor("dinv", [P, NGROUP], f32, kind="ExternalInput")
    W1_t = nc.dram_tensor("W1", [IN_C, HID], f32, kind="ExternalInput")
    W2_t = nc.dram_tensor("W2", [HID, OUT_C], f32, kind="ExternalInput")
    b1b_t = nc.dram_tensor("b1b", [P, HID], f32, kind="ExternalInput")
    b2b_t = nc.dram_tensor("b2b", [P, OUT_C], f32, kind="ExternalInput")
    out_t = nc.dram_tensor("out", [NPAD, OUT_C], f32, kind="ExternalOutput")

    hs1_own = nc.dram_tensor("hs1_own", [NPAD, HID], f32)
    hs2_own = nc.dram_tensor("hs2_own", [NPAD, OUT_C], f32)
    table1 = nc.dram_tensor("table1", [VTOT, HID], f32, addr_space="Shared")
    table2 = nc.dram_tensor("table2", [VTOT, OUT_C], f32, addr_space="Shared")

    groups = list(range(NGROUP))
    rg = [list(range(NCORES))]

    with tile.TileContext(nc) as tc:
        with tc.tile_pool(name="const", bufs=1) as cp, \
             tc.tile_pool(name="xt", bufs=4) as xp, \
             tc.tile_pool(name="gat", bufs=12) as gp, \
             tc.tile_pool(name="work", bufs=4) as wp, \
             tc.tile_pool(name="ps1", bufs=2, space="PSUM") as ps1, \
             tc.tile_pool(name="psT", bufs=2, space="PSUM") as psT, \
             tc.tile_pool(name="ps2", bufs=2, space="PSUM") as ps2:

            ident = cp.tile([P, P], f32)
            make_identity(nc, ident[:])
            w1a = cp.tile([P, HID], f32)
            w1b = cp.tile([P, HID], f32)
            nc.sync.dma_start(out=w1a[:], in_=W1_t[0:P, :])
            nc.sync.dma_start(out=w1b[:], in_=W1_t[P:IN_C, :])
            w2s = cp.tile([HID, OUT_C], f32)
            nc.sync.dma_start(out=w2s[:], in_=W2_t[:, :])
            b1s = cp.tile([P, HID], f32)
            nc.sync.dma_start(out=b1s[:], in_=b1b_t[:, :])
            b2s = cp.tile([P, OUT_C], f32)
            nc.sync.dma_start(out=b2s[:], in_=b2b_t[:, :])
            dvs = cp.tile([P, NGROUP], f32)
            nc.sync.dma_start(out=dvs[:], in_=dinv_t[:, :])
            offs_sb = cp.tile([P, sumD], i32)
            nc.sync.dma_start(out=offs_sb[:], in_=offs_t[:, :])

            # ---- P1: project own rows, scatter into shared table1 ----
            for _rep in range(repeat):
              for g in groups:
                  xt0 = xp.tile([P, P], f32, tag="xt0")
                  xt1 = xp.tile([P, P], f32, tag="xt1")
                  nc.sync.dma_start(out=xt0[:], in_=xT_t[0:P, g * P:(g + 1) * P])
                  nc.sync.dma_start(out=xt1[:], in_=xT_t[P:IN_C, g * P:(g + 1) * P])
                  pm = ps1.tile([P, HID], f32)
                  nc.tensor.matmul(out=pm[:], lhsT=xt0[:], rhs=w1a[:],
                                   start=True, stop=False)
                  nc.tensor.matmul(out=pm[:], lhsT=xt1[:], rhs=w1b[:],
                                   start=False, stop=True)
                  hs1 = wp.tile([P, HID], f32, tag="hs1")
                  nc.vector.tensor_scalar_mul(hs1[:], pm[:], dvs[:, g:g + 1])
                  nc.sync.dma_start(out=hs1_own[g * P:(g + 1) * P, :], in_=hs1[:])

              nc.gpsimd.collective_compute(
                  "AllGather", mybir.AluOpType.bypass, replica_groups=rg,
                  ins=[hs1_own[:, :]], outs=[table1[:, :]])

              # ---- A1 + L2 projection, per group ----
              NACC = 8
              for g in groups:
                  D = int(Dg[g])
                  nacc = min(NACC, D)
                  # self-loop term: own hs1 rows, no gather needed
                  sf1 = wp.tile([P, HID], f32, tag="sf1")
                  nc.sync.dma_start(out=sf1[:],
                                    in_=hs1_own[g * P:(g + 1) * P, :])
                  accs = [gp.tile([P, HID], f32, tag=f"a1_{k}", name=f"a1g{g}_{k}") for k in range(nacc)]
                  for j in range(D):
                      col = int(cumD[g]) + j
                      gi = nc.gpsimd.indirect_dma_start(
                          out=accs[j % nacc][:],
                          out_offset=None,
                          in_=table1[:, :],
                          in_offset=bass.IndirectOffsetOnAxis(
                              ap=offs_sb[:, col:col + 1], axis=0),
                          compute_op=(mybir.AluOpType.bypass if j < nacc
                                      else mybir.AluOpType.add),
                      )
                  accs.append(sf1)
                  red = wp.tile([P, HID], f32, tag="red1")
                  while len(accs) > 2:
                      t = wp.tile([P, HID], f32, tag="cmb1")
                      nc.vector.tensor_tensor(out=t[:], in0=accs[0][:],
                                              in1=accs[1][:],
                                              op=mybir.AluOpType.add)
                      accs = [t] + accs[2:]
                  nc.vector.tensor_tensor(out=red[:], in0=accs[0][:],
                                          in1=accs[1][:],
                                          op=mybir.AluOpType.add)
                  u = wp.tile([P, HID], f32, tag="u1")
                  nc.vector.tensor_scalar_mul(u[:], red[:], dvs[:, g:g + 1])
                  v = wp.tile([P, HID], f32, tag="v1")
                  nc.vector.tensor_tensor(out=v[:], in0=u[:], in1=b1s[:],
                                          op=mybir.AluOpType.add)
                  h1 = wp.tile([P, HID], f32, tag="h1")
                  nc.scalar.activation(out=h1[:], in_=v[:],
                                       func=mybir.ActivationFunctionType.Relu,
                                       scale=dvs[:, g:g + 1])
                  # L2 projection: hs2 = (dinv*h1) @ W2  (dinv already folded)
                  pT = psT.tile([HID, P], f32)
                  nc.tensor.transpose(out=pT[:], in_=h1[:], identity=ident[:])
                  h1T = wp.tile([HID, P], f32, tag="h1T")
                  nc.vector.tensor_copy(out=h1T[:], in_=pT[:])
                  pm2 = ps2.tile([P, OUT_C], f32)
                  nc.tensor.matmul(out=pm2[:], lhsT=h1T[:], rhs=w2s[:],
                                   start=True, stop=True)
                  hs2 = wp.tile([P, OUT_C], f32, tag="hs2")
                  nc.vector.tensor_copy(out=hs2[:], in_=pm2[:])
                  nc.sync.dma_start(out=hs2_own[g * P:(g + 1) * P, :], in_=hs2[:])

              nc.gpsimd.collective_compute(
                  "AllGather", mybir.AluOpType.bypass, replica_groups=rg,
                  ins=[hs2_own[:, :]], outs=[table2[:, :]])

              # ---- A2: final aggregation ----
              for g in groups:
                  D = int(Dg[g])
                  nacc = min(NACC, D)
                  sf2 = wp.tile([P, OUT_C], f32, tag="sf2")
                  nc.sync.dma_start(out=sf2[:],
                                    in_=hs2_own[g * P:(g + 1) * P, :])
                  accs = [gp.tile([P, OUT_C], f32, tag=f"a2_{k}", name=f"a2g{g}_{k}") for k in range(nacc)]
                  for j in range(D):
                      col = int(cumD[g]) + j
                      gi2 = nc.gpsimd.indirect_dma_start(
                          out=accs[j % nacc][:],
                          out_offset=None,
                          in_=table2[:, :],
                          in_offset=bass.IndirectOffsetOnAxis(
                              ap=offs_sb[:, col:col + 1], axis=0),
                          compute_op=(mybir.AluOpType.bypass if j < nacc
                                      else mybir.AluOpType.add),
                      )
                  accs.append(sf2)
                  red2 = wp.tile([P, OUT_C], f32, tag="red2")
                  while len(accs) > 2:
                      t = wp.tile([P, OUT_C], f32, tag="cmb2")
                      nc.vector.tensor_tensor(out=t[:], in0=accs[0][:],
                                              in1=accs[1][:],
                                              op=mybir.AluOpType.add)
                      accs = [t] + accs[2:]
                  nc.vector.tensor_tensor(out=red2[:], in0=accs[0][:],
                                          in1=accs[1][:],
                                          op=mybir.AluOpType.add)
                  u2 = wp.tile([P, OUT_C], f32, tag="u2")
                  nc.vector.tensor_scalar_mul(u2[:], red2[:], dvs[:, g:g + 1])
                  o2 = wp.tile([P, OUT_C], f32, tag="o2")
                  nc.vector.tensor_tensor(out=o2[:], in0=u2[:], in1=b2s[:],
                                          op=mybir.AluOpType.add)
                  nc.sync.dma_start(out=out_t[g * P:(g + 1) * P, :], in_=o2[:])

    nc.compile()
    _NC_CACHE[key] = nc
    return nc


def kernel(x, edge_index, W1, b1, W2, b2):
    from concourse.bass_utils import run_bass_kernel_spmd

    prep = _host_prep(x, edge_index, W1, b1, W2, b2)
    nc = _build_bass(prep["Dg"], prep["sumD"])

    in_maps = []
    for c in range(NCORES):
        in_maps.append({
            "xT": prep["xT"][c],
            "offs": np.ascontiguousarray(prep["offs"][c]),
            "dinv": prep["dinv"][c],
            "W1": prep["W1"],
            "W2": prep["W2"],
            "b1b": prep["b1b"],
            "b2b": prep["b2b"],
        })
    import time as _time
    res = None
    for attempt in range(3):
        try:
            res = run_bass_kernel_spmd(nc, in_maps, core_ids=list(range(NCORES)))
            break
        except Exception:
            if attempt == 2:
                raise
            _time.sleep(15.0)
    assert res is not None

    out = np.empty((N, OUT_C), dtype=np.float32)
    for c in range(NCORES):
        out[prep["perms"][c]] = res.results[c]["out"][:NPC]
    return out


if __name__ == "__main__":
    rng = np.random.default_rng(0)
    x = rng.standard_normal((N, IN_C)).astype(np.float32)
    ei = rng.integers(0, N, size=(2, 3200000)).astype(np.int64)
    W1 = rng.standard_normal((IN_C, HID)).astype(np.float32) / 16.0
    W2 = rng.standard_normal((HID, OUT_C)).astype(np.float32) / 5.66
    out = kernel(x, ei, W1, np.zeros(HID, np.float32), W2,
                 np.zeros(OUT_C, np.float32))
    print(out.shape, out.dtype, np.abs(out).mean())

